# revision 1
# baseline (speedup 1.0000x reference)
"""APPNP (GCN-normalized propagation) distributed Bass kernel for 8 TRN2 NeuronCores.

Algorithm (matches reference):
    h  = relu(x @ W1 + b1) @ W2 + b2            # MLP encoder
    A_hat = D^-1/2 (A + I) D^-1/2               # GCN norm with self-loops
    z_{k+1} = 0.9 * A_hat z_k + 0.1 * h, K=16   # APPNP propagation
    out = log_softmax(z_K)

Key identity: edge (s -> d) weight = dinv[s]*dinv[d]; with zs = dinv * z the
aggregation is agg[d] = dinv[d] * sum_{s->d} zs[s] + dinv[d]^2 * z[d] (self
loop). Per-edge work is pure 256-byte-row gathers; all scaling is node-local.

Sharding: destination-node sharding across 8 cores (12500 nodes each).
Per step, per core:
  1. zs = dinv*z -> DMA -> AllGather -> full zs in core-local DRAM.
  2. Per src-range r (4 ranges of <=25088 padded rows so indices fit int16):
     stripe gathers. Stripe j holds the j-th in-edge of every destination,
     destinations ordered per SBUF partition by descending range-degree, so
     stripe width shrinks with j (padding points at an always-zero row).
     DVE adds accumulate stripes into S_r (sorted layout); S_r is scaled by
     c1 (sorted) and written to DRAM.
  3. Merge: per range, one more gather (unique indices = inverse sort
     permutation) brings S_r back in canonical slot order; Z += it.
     z-update: Z = c2*Z + alpha*H + sum_r merge_r.
NOTE: dma_scatter_add is NOT used anywhere: its HBM read-modify-write loses
updates for duplicate destination rows (verified on HW), so the whole
aggregation is built from gathers + on-chip adds only.

Host-side work is limited to graph-structure preprocessing (degree counts,
index layout/int16 conversion, sharding, padding) -- all feature FLOPs run
on device.
"""
import sys
import os

sys.path.insert(0, "/opt/trn_rl_repo")

import numpy as np

# ---------------------------------------------------------------- config

P = 128


class Cfg:
    def __init__(self, n_nodes, f_in, f_h, f_out, k_steps, alpha, n_cores,
                 range_cores, ch):
        self.N = n_nodes
        self.F_IN = f_in
        self.F_H = f_h
        self.F_OUT = f_out
        self.K = k_steps
        self.ALPHA = alpha
        self.NC = n_cores
        self.NSH = n_nodes // n_cores              # real nodes per core
        # always at least one pad row (stripe-padding gather target)
        self.NT = (self.NSH + 1 + P - 1) // P      # tiles per core
        self.NPAD = self.NT * P
        self.RANGE_CORES = range_cores             # cores per int16 gather range
        self.RANGE_ROWS = range_cores * self.NPAD
        assert self.RANGE_ROWS <= 32768
        self.NRANGE = n_cores // range_cores
        self.CH = ch                               # idx per gather call
        assert ch % P == 0
        self.PAD_SLOT = self.NPAD - 1              # a pad row (node id NPAD-1 >= NSH)


REAL = Cfg(n_nodes=100000, f_in=512, f_h=128, f_out=64, k_steps=16,
           alpha=0.1, n_cores=8, range_cores=2, ch=8192)

IDX_BLOCK = 2      # chunk-pairs of stripe indices per idx-stream DMA


# ---------------------------------------------------------------- host prep

def _slot_of(ids, cfg):
    """local node id -> DRAM row ('slot') in the [NPAD, F] per-core buffers.

    SBUF tiles are [128, NT, F] with node v at [v % 128, v // 128]; a plain
    contiguous DMA maps partition p to rows [p*NT, (p+1)*NT), so
    slot(v) = (v % 128) * NT + v // 128.
    """
    return (ids % P) * cfg.NT + ids // P


def _pack_idx(a):
    """int16 idx array [n] -> [128, n/16] ucode layout.

    Element (c, j) = idx[j*16 + c]; the 16-row block is replicated to all
    128 partitions (each SWDGE queue's Q7 pair reads its own 32-row group).
    """
    n = a.shape[0]
    assert n % 16 == 0
    b = a.reshape(n // 16, 16).T.astype(np.int16)  # [16, n/16]
    return np.tile(b, (8, 1))


class Structure:
    """Shared (cross-core identical) stripe structure for one problem."""

    def __init__(self, widths, chunks, cfg):
        self.widths = widths            # widths[r] = list of stripe widths T_j
        self.chunks = chunks            # chunks[r] = number of CH-idx calls
        self.cfg = cfg
        # per range: len_r (valid idx count), call segments
        self.len_r = [128 * sum(w) for w in widths]
        self.segments = []              # [r][call] -> list of (u0,u1,s0,copy)
        for r in range(cfg.NRANGE):
            segs_per_call = [[] for _ in range(chunks[r])]
            pos = 0                     # in t-units of 128 idx
            for j, T in enumerate(widths[r]):
                done = 0
                while done < T:
                    call = pos // (cfg.CH // P)
                    u0 = pos % (cfg.CH // P)
                    take = min(T - done, cfg.CH // P - u0)
                    segs_per_call[call].append(
                        (u0, u0 + take, done, j == 0))
                    pos += take
                    done += take
            self.segments.append(segs_per_call)

    def key(self):
        return (tuple(tuple(w) for w in self.widths), tuple(self.chunks))


def host_prep(x, edge_index, W1, b1, W2, b2, cfg):
    """Build per-core input maps + shared structure."""
    x = np.asarray(x, np.float32)
    ei = np.asarray(edge_index)
    W1 = np.asarray(W1, np.float32)
    b1 = np.asarray(b1, np.float32)
    W2 = np.asarray(W2, np.float32)
    b2 = np.asarray(b2, np.float32)
    src, dst = ei[0].astype(np.int64), ei[1].astype(np.int64)
    NR, NSH, NT, CH = cfg.NRANGE, cfg.NSH, cfg.NT, cfg.CH

    deg_tot = np.bincount(dst, minlength=cfg.N).astype(np.float32) + 1.0
    dinv = 1.0 / np.sqrt(deg_tot)

    ids = np.arange(NSH, dtype=np.int64)
    slot_arr = _slot_of(ids, cfg)                  # local id -> slot

    # padded-global row of each edge src (row in the AllGathered zs buffer)
    g_core = src // NSH
    g_row = g_core * cfg.NPAD + _slot_of(src - g_core * NSH, cfg)
    g_range = g_row // cfg.RANGE_ROWS
    g_idx = (g_row - g_range * cfg.RANGE_ROWS).astype(np.int16)
    d_core = dst // NSH

    # ---- per (core, range): degrees, per-partition sorted orders ----
    # deg[c][r][v_local], order t'_r(v)
    per_cr = {}
    for c in range(cfg.NC):
        mc = d_core == c
        dl = dst[mc] - c * NSH
        gr = g_range[mc]
        gi = g_idx[mc]
        for r in range(NR):
            mr = gr == r
            per_cr[(c, r)] = (dl[mr], gi[mr])

    # per-partition sorted order and stripe widths
    # partition of local id v is v % 128; its t-list is ids p,p+128,...
    tsort = {}        # (c,r) -> trank[NPAD local id] = sorted position t'
    degs = {}
    widths = [[] for _ in range(NR)]
    for r in range(NR):
        # common stripe widths = max over cores of per-core T_j
        percore_counts = []
        for c in range(cfg.NC):
            dl, _ = per_cr[(c, r)]
            deg = np.bincount(dl, minlength=cfg.NPAD)  # pad ids deg 0
            degs[(c, r)] = deg
            # sort within partitions by -deg
            trank = np.empty(cfg.NPAD, np.int64)
            for p in range(P):
                pid = np.arange(p, cfg.NPAD, P)    # ids in partition p
                order = np.argsort(-deg[pid], kind="stable")
                trank[pid[order]] = np.arange(len(pid))
            tsort[(c, r)] = trank
            # count_p(j) = #dests in partition with deg >= j ->
            # T_j = max_p count_p(j)
            dmax = int(deg.max()) if deg.size else 0
            Tj = []
            for j in range(1, dmax + 1):
                cnt = np.zeros(P, np.int64)
                m = deg >= j
                np.add.at(cnt, np.nonzero(m)[0] % P, 1)
                Tj.append(int(cnt.max()))
            percore_counts.append(Tj)
        jmax = max(len(t) for t in percore_counts)
        for j in range(jmax):
            widths[r].append(max(t[j] for t in percore_counts if len(t) > j))
        if widths[r]:
            # stripe 1 full width: its copy must initialize ALL of S
            # (zero-degree dests read the always-zero pad row)
            widths[r][0] = NT

    chunks = tuple(-(-128 * sum(widths[r]) // CH) for r in range(NR))
    st = Structure([list(w) for w in widths], chunks, cfg)

    in_maps = []
    for c in range(cfg.NC):
        # ---- stripe gather indices ----
        stripe_blocks = []
        merge_idx = np.empty((NR, cfg.NPAD), np.int16)
        c1s_all = np.empty((NR, P, NT), np.float32)
        dv = np.zeros(cfg.NPAD, np.float32)
        dv[slot_arr] = dinv[c * NSH:(c + 1) * NSH]
        # dv indexed by slot; also need by local id:
        dvi = np.zeros(cfg.NPAD, np.float32)
        dvi[:NSH] = dinv[c * NSH:(c + 1) * NSH]

        for r in range(NR):
            dl, gi = per_cr[(c, r)]
            deg = degs[(c, r)]
            trank = tsort[(c, r)]
            # edge rank j within its dest: stable sort by dest
            order = np.argsort(dl, kind="stable")
            dls, gis = dl[order], gi[order]
            first = np.zeros(len(dls), np.int64)
            if len(dls):
                newd = np.empty(len(dls), bool)
                newd[0] = True
                newd[1:] = dls[1:] != dls[:-1]
                idxs = np.arange(len(dls))
                first = idxs - np.maximum.accumulate(np.where(newd, idxs, 0))
            jrank = first                           # 0-based depth
            # stripe start offsets (t-units)
            W = st.widths[r]
            starts = np.concatenate([[0], np.cumsum(W)])[:-1]
            # position of edge = (starts[j] + trank[dest]) * 128 + dest%128
            pos = (starts[jrank] + trank[dls]) * P + (dls % P)
            L = st.chunks[r] * CH
            arr = np.full(L, -1, np.int16)
            valid = st.len_r[r]
            # in-stripe padding -> PAD row of core 2r's shard (always zero)
            arr[:valid] = cfg.PAD_SLOT
            arr[pos] = gis
            stripe_blocks.append(arr)
            # merge gather idx: logical i = t*128+p -> row p*NT + t'(id(p,t))
            # id at (p, t) canonical = t*128 + p
            ii = np.arange(cfg.NPAD)
            pp, tt = ii % P, ii // P               # canonical (p, t)
            merge_idx[r] = (pp * NT + trank[tt * P + pp]).astype(np.int16)
            # c1 sorted layout: position (p, t') holds c1 of that dest
            c1v = (1.0 - cfg.ALPHA) * dvi          # by local id
            ids_all = np.arange(cfg.NPAD)
            c1s = np.empty(cfg.NPAD, np.float32)
            c1s[(ids_all % P) * NT + trank[ids_all]] = c1v[ids_all]
            c1s_all[r] = c1s.reshape(P, NT)

        idx_all = np.concatenate(stripe_blocks)
        TOTCH = sum(st.chunks)
        idx_blocks = np.stack([
            _pack_idx(idx_all[k * CH:(k + 1) * CH]) for k in range(TOTCH)])

        xs = x[c * NSH:(c + 1) * NSH]
        xp = np.zeros((cfg.NPAD, cfg.F_IN), np.float32)
        xp[:NSH] = xs
        xT = np.ascontiguousarray(xp.T)

        dv2 = (1.0 - cfg.ALPHA) * dv * dv          # slot layout
        mi = np.stack([_pack_idx(merge_idx[r]) for r in range(NR)])

        in_maps.append({
            "xT": xT,
            "W1": W1.astype(np.float32),
            "W2": W2.astype(np.float32),
            "b1": b1.reshape(cfg.F_H, 1).astype(np.float32),
            "b2r": np.tile(b2.reshape(1, cfg.F_OUT), (P, 1)).astype(np.float32),
            "dinv": dv.reshape(P, NT),
            "c2": dv2.reshape(P, NT),
            "c1s": c1s_all,                        # [NR, 128, NT]
            "idx": idx_blocks,                     # [TOTCH, 128, CH/16]
            "midx": mi,                            # [NR, 128, NPAD/16]
        })
    return in_maps, st


# ---------------------------------------------------------------- builder

def _patch_tile_queue_assignment():
    """Distribute SWDGE gathers over the 4 Q7 queue-pairs.

    Tile assigns DMASW sem lanes round-robin in *scheduled* order and the
    sim/HW lock each sem to one queue, so queue_num must be a function of
    the lane. Patch the tick assigner to set queue_num = lane %% 4 right
    where the lane is chosen.
    """
    from concourse import tile_sem_assignment as tsa
    import concourse.mybir as mybir
    if getattr(tsa.TileClockTick, "_qpatched", False):
        return
    orig = tsa.TileClockTick._assign_tick

    def patched(self, inst):
        if isinstance(inst, (mybir.InstDMAGatherAnt,
                             mybir.InstDMAScatterAddAnt)):
            inst.queue_num = self.next_sw_dma_idx % 4
        return orig(self, inst)

    tsa.TileClockTick._assign_tick = patched
    tsa.TileClockTick._qpatched = True


def build_bass(cfg, st, debug=False):
    import concourse.bass as bass
    import concourse.bacc as bacc
    import concourse.mybir as mybir
    import concourse.tile as tile

    _patch_tile_queue_assignment()

    f32 = mybir.dt.float32
    i16 = mybir.dt.int16
    NT, F, FH, FI = cfg.NT, cfg.F_OUT, cfg.F_H, cfg.F_IN
    KC = FI // P
    TOTCH = sum(st.chunks)
    IC = cfg.CH // 16                              # idx cols per chunk
    NR = cfg.NRANGE

    nc = bacc.Bacc("TRN2", target_bir_lowering=False, debug=debug,
                   num_devices=cfg.NC, num_swdge_queues=4,
                   dynamic_dma_scratch_size=32768)

    xT_d = nc.dram_tensor("xT", [FI, cfg.NPAD], f32, kind="ExternalInput")
    w1_d = nc.dram_tensor("W1", [FI, FH], f32, kind="ExternalInput")
    w2_d = nc.dram_tensor("W2", [FH, F], f32, kind="ExternalInput")
    b1_d = nc.dram_tensor("b1", [FH, 1], f32, kind="ExternalInput")
    b2_d = nc.dram_tensor("b2r", [P, F], f32, kind="ExternalInput")
    dinv_d = nc.dram_tensor("dinv", [P, NT], f32, kind="ExternalInput")
    c2_d = nc.dram_tensor("c2", [P, NT], f32, kind="ExternalInput")
    c1s_d = nc.dram_tensor("c1s", [NR, P, NT], f32, kind="ExternalInput")
    idx_d = nc.dram_tensor("idx", [TOTCH, P, IC], i16, kind="ExternalInput")
    midx_d = nc.dram_tensor("midx", [NR, P, cfg.NPAD // 16], i16,
                            kind="ExternalInput")
    out_d = nc.dram_tensor("out", [cfg.NPAD, F], f32, kind="ExternalOutput")

    zsb_d = nc.dram_tensor("zs_bounce", [cfg.NPAD, F], f32)
    zsfull_d = nc.dram_tensor("zs_full", [cfg.NC * cfg.NPAD, F], f32,
                              addr_space="Shared")
    sd_d = [nc.dram_tensor(f"sd{r}", [cfg.NPAD, F], f32) for r in range(NR)]

    zsb_v = zsb_d[:].rearrange("(p t) f -> p t f", p=P)
    out_v = out_d[:].rearrange("(p t) f -> p t f", p=P)
    sd_v = [s[:].rearrange("(p t) f -> p t f", p=P) for s in sd_d]
    xT_v = xT_d[:].rearrange("(k p) n -> p k n", p=P)
    w1_v = w1_d[:].rearrange("(k p) m -> p k m", p=P)

    groups = [list(range(cfg.NC))]
    add = mybir.AluOpType.add
    mult = mybir.AluOpType.mult

    with tile.TileContext(nc) as tc:
        with tc.tile_pool(name="const", bufs=1) as cpool, \
             tc.tile_pool(name="state", bufs=1) as spool:
            w2sb = cpool.tile([P, F], f32)
            nc.sync.dma_start(w2sb[:], w2_d[:])
            b1sb = cpool.tile([P, 1], f32)
            nc.sync.dma_start(b1sb[:], b1_d[:])
            b2sb = cpool.tile([P, F], f32)
            nc.sync.dma_start(b2sb[:], b2_d[:])
            dinvsb = cpool.tile([P, NT], f32)
            nc.sync.dma_start(dinvsb[:], dinv_d[:])
            c2sb = cpool.tile([P, NT], f32)
            nc.sync.dma_start(c2sb[:], c2_d[:])
            c1sb = cpool.tile([P, NR, NT], f32)
            nc.sync.dma_start(c1sb[:], c1s_d[:].rearrange("r p t -> p r t"))
            misb = cpool.tile([P, NR, cfg.NPAD // 16], i16)
            nc.sync.dma_start(misb[:], midx_d[:].rearrange("r p c -> p r c"))

            Z = spool.tile([P, NT, F], f32)
            H = spool.tile([P, NT, F], f32)

            # ---------------- MLP encoder ----------------
            RB = 512
            with tc.tile_pool(name="mlpx", bufs=2) as xpool, \
                 tc.tile_pool(name="mlph", bufs=2) as hpool, \
                 tc.tile_pool(name="mlpw", bufs=1) as wpool, \
                 tc.tile_pool(name="mlpp", bufs=2, space="PSUM") as ppool, \
                 tc.tile_pool(name="mlpp2", bufs=2, space="PSUM") as p2pool:
                w1sb = wpool.tile([P, KC, FH], f32)
                nc.sync.dma_start(w1sb[:], w1_v[:])
                nblk = (cfg.NPAD + RB - 1) // RB
                for j in range(nblk):
                    r0 = j * RB
                    rb = min(RB, cfg.NPAD - r0)
                    xt = xpool.tile([P, KC, RB], f32, tag="xt")
                    nc.sync.dma_start(xt[:, :, :rb], xT_v[:, :, r0:r0 + rb])
                    ps1 = ppool.tile([P, RB], f32, tag="ps1")
                    for kc in range(KC):
                        nc.tensor.matmul(ps1[:, :rb], w1sb[:, kc, :],
                                         xt[:, kc, :rb],
                                         start=(kc == 0), stop=(kc == KC - 1))
                    h1 = hpool.tile([P, RB], f32, tag="h1")
                    nc.scalar.activation(h1[:, :rb], ps1[:, :rb],
                                         mybir.ActivationFunctionType.Relu,
                                         bias=b1sb[:, :])
                    for i in range(rb // P):
                        t = (r0 + i * P) // P
                        ps2 = p2pool.tile([P, F], f32, tag="ps2")
                        nc.tensor.matmul(ps2[:], h1[:, i * P:(i + 1) * P],
                                         w2sb[:], start=True, stop=True)
                        nc.vector.tensor_tensor(Z[:, t, :], ps2[:], b2sb[:],
                                                op=add)
            nc.vector.tensor_copy(H[:], Z[:])

            # ---------------- propagation ----------------
            with tc.tile_pool(name="zsp", bufs=1) as zspool, \
                 tc.tile_pool(name="sp", bufs=2) as sgpool, \
                 tc.tile_pool(name="val", bufs=2) as vpool, \
                 tc.tile_pool(name="idxp", bufs=3) as ipool:
                dinv_b = dinvsb[:].to_broadcast([P, NT, F])
                c2_b = c2sb[:].to_broadcast([P, NT, F])

                for step in range(cfg.K):
                    ZS = zspool.tile([P, NT, F], f32, tag="zs")
                    nc.vector.tensor_tensor(ZS[:], Z[:], dinv_b, op=mult)
                    nc.sync.dma_start(zsb_v[:], ZS[:])
                    nc.gpsimd.collective_compute(
                        "AllGather", mybir.AluOpType.bypass,
                        replica_groups=groups,
                        ins=[zsb_d[:]], outs=[zsfull_d[:]])

                    # z pre-update (overlaps gathers): Z = c2*Z + alpha*H
                    nc.vector.tensor_tensor(Z[:], Z[:], c2_b, op=mult)
                    nc.vector.scalar_tensor_tensor(
                        Z[:], H[:], float(cfg.ALPHA), Z[:],
                        op0=mult, op1=add)

                    # stripe idx streaming for the whole step
                    itiles = {}
                    for b0 in range(0, TOTCH, IDX_BLOCK):
                        bn = min(IDX_BLOCK, TOTCH - b0)
                        it = ipool.tile([P, IDX_BLOCK, IC], i16, tag="idx")
                        nc.sync.dma_start(
                            it[:, :bn, :],
                            idx_d[b0:b0 + bn].rearrange("b p c -> p b c"))
                        for k in range(bn):
                            itiles[b0 + k] = (it, k)

                    ck = 0
                    for r in range(NR):
                        if st.chunks[r] == 0:
                            continue
                        S = sgpool.tile([P, NT, F], f32, tag="s")
                        src_ap = zsfull_d[r * cfg.RANGE_ROWS:
                                          (r + 1) * cfg.RANGE_ROWS, :]
                        for k in range(st.chunks[r]):
                            it, kk = itiles[ck]
                            nvalid = min(cfg.CH, st.len_r[r] - k * cfg.CH)
                            val = vpool.tile([P, cfg.CH // P, F], f32,
                                             tag="val")
                            nc.gpsimd.dma_gather(
                                val[:], src_ap, it[:, kk, :],
                                cfg.CH, nvalid, F, single_packet=False)
                            for (u0, u1, s0, copy) in st.segments[r][k]:
                                dstap = S[:, s0:s0 + (u1 - u0), :]
                                if copy:
                                    nc.vector.tensor_copy(
                                        dstap, val[:, u0:u1, :])
                                else:
                                    nc.vector.tensor_tensor(
                                        dstap, dstap, val[:, u0:u1, :],
                                        op=add)
                            ck += 1
                        # scale by c1 (sorted layout) and park in DRAM
                        nc.vector.tensor_tensor(
                            S[:], S[:],
                            c1sb[:, r, :].to_broadcast([P, NT, F]), op=mult)
                        nc.sync.dma_start(sd_v[r][:], S[:])
                    # merges LAST: keeps the in-order Pool engine from
                    # stalling between ranges on the Sd write chain
                    for r in range(NR):
                        if st.chunks[r] == 0:
                            continue
                        G = sgpool.tile([P, NT, F], f32, tag="s")
                        nc.gpsimd.dma_gather(
                            G[:], sd_d[r][:], misb[:, r, :],
                            cfg.NPAD, cfg.NPAD, F, single_packet=False)
                        nc.vector.tensor_tensor(Z[:], Z[:], G[:], op=add)

                # ---------------- log_softmax ----------------
                M = cpool.tile([P, NT], f32)
                nc.vector.tensor_reduce(M[:], Z[:], axis=mybir.AxisListType.X,
                                        op=mybir.AluOpType.max)
                ZC = zspool.tile([P, NT, F], f32, tag="zs")
                nc.vector.tensor_tensor(ZC[:], Z[:],
                                        M[:].to_broadcast([P, NT, F]),
                                        op=mybir.AluOpType.subtract)
                EX = sgpool.tile([P, NT, F], f32, tag="s")
                nc.scalar.activation(EX[:], ZC[:],
                                     mybir.ActivationFunctionType.Exp)
                S2 = cpool.tile([P, NT], f32)
                nc.vector.tensor_reduce(S2[:], EX[:], axis=mybir.AxisListType.X,
                                        op=add)
                LS = cpool.tile([P, NT], f32)
                nc.scalar.activation(LS[:], S2[:],
                                     mybir.ActivationFunctionType.Ln)
                nc.vector.tensor_tensor(ZC[:], ZC[:],
                                        LS[:].to_broadcast([P, NT, F]),
                                        op=mybir.AluOpType.subtract)
                nc.sync.dma_start(out_v[:], ZC[:])

    nc.compile()
    return nc


# ---------------------------------------------------------------- runner

_CACHE = {}
LAST_EXEC_NS = None
LAST_RESULTS = None


def _install_ntff_hook():
    """Shim antenv.axon_hooks (absent from the container's antenv stub) and
    register the ctypes NTFF profile hook so trace=True yields exec_time_ns."""
    try:
        from antenv.axon_hooks import get_axon_ntff_profile_hook  # noqa: F401
        return
    except ImportError:
        pass
    import antenv
    shim = "/tmp/antenv_shim"
    os.makedirs(shim, exist_ok=True)
    with open(os.path.join(shim, "axon_hooks.py"), "w") as f:
        f.write(
            "_h = None\n"
            "def set_axon_ntff_profile_hook(h):\n"
            "    global _h\n    _h = h\n"
            "def get_axon_ntff_profile_hook():\n"
            "    return _h\n")
    antenv.__path__.append(shim)
    try:
        from trn_agent_boot.trn_boot import _ntff_profile_via_ctypes
        from antenv.axon_hooks import set_axon_ntff_profile_hook
        set_axon_ntff_profile_hook(
            _ntff_profile_via_ctypes("/opt/axon/libaxon_pjrt.so"))
    except Exception as e:  # degrade: run without HW timing
        print(f"ntff hook install failed: {e}")
    import concourse.bass_utils as bu
    bu.upload_artifacts = lambda tmpdir: tmpdir


def _get_nc(cfg, st):
    key = (id(cfg), st.key())
    if key not in _CACHE:
        _CACHE[key] = build_bass(cfg, st)
    return _CACHE[key]


def run(inputs, cfg=REAL, trace=False, trace_kwargs=None):
    global LAST_EXEC_NS, LAST_RESULTS
    from concourse.bass_utils import run_bass_kernel_spmd

    if trace:
        _install_ntff_hook()
    in_maps, st = host_prep(
        inputs["x"], inputs["edge_index"], inputs["W1"], inputs["b1"],
        inputs["W2"], inputs["b2"], cfg)
    nc = _get_nc(cfg, st)
    res = run_bass_kernel_spmd(nc, in_maps, list(range(cfg.NC)),
                               trace=trace, **(trace_kwargs or {}))
    LAST_EXEC_NS = res.exec_time_ns
    LAST_RESULTS = res
    outs = []
    sl = _slot_of(np.arange(cfg.NSH), cfg)
    for c in range(cfg.NC):
        o = res.results[c]["out"]                  # [NPAD, F] in slot order
        outs.append(o[sl])
    return np.concatenate(outs, axis=0)


def kernel(**inputs):
    return run(inputs, REAL, trace=False)

